# revision 10
# baseline (speedup 1.0000x reference)
"""Trainium2 Bass kernel for nn_BigramLanguageModel (8-layer GPT, B=4, T=1024,
E=1024, H=16, V=50257) on 8 NeuronCores.

Sharding: DP4 x SP2.  Core pair (2i, 2i+1) owns batch element i; within the
pair core A holds tokens [0:512), core B holds [512:1024).  Per layer each
pair AllGathers k/v (2-rank collectives, overlapped with compute); LN / QKV /
attention / FFN are token-local.  The lm_head is vocab-sharded 8 ways after an
8-way AllGather of the final hidden states; each core also emits per-shard
softmax denominators so the host can finish the loss cheaply.

Activations are kept feature-major in SBUF ([features -> partitions,
tokens -> free]); LN statistics and softmax denominators are computed with
ones-matmuls on the tensor engine; causality comes from per-core mask inputs
so all 8 cores run one uniform SPMD program.
"""

import sys
import os

for _p in ("/opt/trn_rl_repo",):
    if _p not in sys.path and os.path.isdir(_p):
        sys.path.insert(0, _p)

from contextlib import ExitStack
from dataclasses import dataclass

import numpy as np
import ml_dtypes

import concourse.bass as bass
import concourse.mybir as mybir
import concourse.tile as tile
from concourse import bacc
from concourse import bass_utils

BF16 = mybir.dt.bfloat16
F32 = mybir.dt.float32
P = 128
D = 64          # head dim
NCORES = 8
PAIRS = [[0, 1], [2, 3], [4, 5], [6, 7]]
ALLC = [list(range(NCORES))]
EPS = 1e-5
PAD_BIAS = -10000.0  # lm bias on padded vocab columns: exp() underflows to 0


@dataclass(frozen=True)
class Cfg:
    V: int = 50257
    T: int = 1024
    E: int = 1024
    H: int = 16
    L: int = 8
    FF: int = 4096
    VSH: int = 6400   # padded vocab shard per core (multiple of 128)

    @property
    def TLOC(self):
        return self.T // 2

    @property
    def NE(self):
        return self.E // P

    @property
    def NT(self):
        return self.TLOC // P

    @property
    def NKV(self):
        return self.T // P

    @property
    def NF(self):
        return self.FF // P

    @property
    def NV(self):
        return self.VSH // P

    @property
    def B(self):
        return 4


FULL = Cfg()


def _f32t(x):
    return np.ascontiguousarray(x, dtype=np.float32)


def _bf16(x):
    return np.ascontiguousarray(np.asarray(x, dtype=np.float32).astype(ml_dtypes.bfloat16))


# --------------------------------------------------------------------------
# device program
# --------------------------------------------------------------------------

def build_body(tc, io, cfg: Cfg):
    """Emit the per-core program. io maps names -> DRAM APs."""
    nc = tc.nc
    NE, NT, NKV, NF, NV = cfg.NE, cfg.NT, cfg.NKV, cfg.NF, cfg.NV
    NH = cfg.H
    TL = cfg.TLOC
    E, FF, VSH = cfg.E, cfg.FF, cfg.VSH
    VW = D + 1            # per-head v block: [ones, v0..v63]
    BT = cfg.B * cfg.T    # all tokens
    NB = BT // TL         # token blocks for the lm head (== 8 cores)
    WCH = min(512, E)     # weight streaming chunk (columns)
    PSW = max(TL, WCH)    # shared psum tile width
    add = mybir.AluOpType.add
    mult = mybir.AluOpType.mult

    # offsets into the packed per-partition constant vector tile
    O_LV = 0
    O_FB1 = O_LV + cfg.L * 6 * NE
    O_LNF = O_FB1 + cfg.L * NF
    O_LMB = O_LNF + 2 * NE
    NCVEC = O_LMB + NV

    with ExitStack() as ctx:
        const = ctx.enter_context(tc.tile_pool(name="const", bufs=1))
        xpool = ctx.enter_context(tc.tile_pool(name="xp", bufs=1))
        hpool = ctx.enter_context(tc.tile_pool(name="hp", bufs=1))
        qkvp = ctx.enter_context(tc.tile_pool(name="qkvp", bufs=1))
        kvall = ctx.enter_context(tc.tile_pool(name="kvall", bufs=1))
        wpool = ctx.enter_context(tc.tile_pool(name="wpool", bufs=3))
        relup = ctx.enter_context(tc.tile_pool(name="relup", bufs=1))
        work = ctx.enter_context(tc.tile_pool(name="work", bufs=6))
        stat = ctx.enter_context(tc.tile_pool(name="stat", bufs=6))
        dramp = ctx.enter_context(tc.tile_pool(name="dramp", bufs=2, space="DRAM"))
        psA = ctx.enter_context(tc.tile_pool(name="psA", bufs=4, space="PSUM"))
        psH = ctx.enter_context(tc.tile_pool(name="psH", bufs=2, space="PSUM"))
        psS = ctx.enter_context(tc.tile_pool(name="psS", bufs=2, space="PSUM"))

        # ---- constants ------------------------------------------------
        onesf = const.tile([P, P], F32)
        nc.vector.memset(onesf[:], 1.0)
        ones_col_f32 = onesf[:, 0:1]
        ones_row_f32 = onesf[0:1, :]
        onesb = const.tile([P, 2], BF16)
        nc.vector.memset(onesb[:], 1.0)
        ones_col_bf = onesb[:, 0:1]

        mask_sb = const.tile([P, NKV, TL], BF16)
        nc.sync.dma_start(mask_sb[:], io["masks"].rearrange("c p t -> p c t"))

        cvec = const.tile([P, NCVEC], F32)
        nc.sync.dma_start(
            cvec[:, O_LV:O_FB1].rearrange("p (l k e) -> p l k e", k=6, e=NE),
            io["lvecs"].rearrange("l k p e -> p l k e"))
        nc.sync.dma_start(
            cvec[:, O_FB1:O_LNF].rearrange("p (l f) -> p l f", f=NF),
            io["fb1"].rearrange("l p f -> p l f"))
        nc.sync.dma_start(
            cvec[:, O_LNF:O_LMB].rearrange("p (k e) -> p k e", e=NE),
            io["lnf"].rearrange("k p e -> p k e"))
        nc.sync.dma_start(cvec[:, O_LMB:O_LMB + NV], io["lmb"][:])

        def lv_ap(l, k, e):
            o = O_LV + (l * 6 + k) * NE + e
            return cvec[:, o:o + 1]

        def fb1_ap(l, f):
            o = O_FB1 + l * NF + f
            return cvec[:, o:o + 1]

        def lnf_ap(k, e):
            o = O_LNF + k * NE + e
            return cvec[:, o:o + 1]

        def lmb_ap(vt):
            o = O_LMB + vt
            return cvec[:, o:o + 1]

        # ---- residual stream ------------------------------------------
        x_sb = xpool.tile([P, NE, TL], F32)
        nc.sync.dma_start(x_sb[:], io["x0"][:])

        # persistent v (token-major, 65-stride head blocks; col 0 of each
        # block stays 1.0 forever and yields softmax denominators for free)
        v_loc = qkvp.tile([P, NT, NH * VW], BF16, tag="vloc")
        v4 = v_loc[:].rearrange("p t (h w) -> p t h w", w=VW)
        nc.vector.memset(v4[:, :, :, D:VW], 1.0)

        # ---- helpers --------------------------------------------------
        def layernorm(src, g_of_e, b_of_e, out_t):
            """src [P,NE,TL] f32 -> out_t [P,NE,TL]; g/b: e -> [P,1] AP.

            stat vector tile rows: 0=mu 1=var 2=mu2 3=rec 4=rstd 5=cc
            """
            ps_sum = psS.tile([1, TL], F32, tag="st")
            for e in range(NE):
                nc.tensor.matmul(ps_sum[:], ones_col_f32, src[:, e, :],
                                 start=(e == 0), stop=(e == NE - 1))
            ps_sq = psS.tile([1, TL], F32, tag="st")
            for e in range(NE):
                sq = work.tile([P, TL], F32, tag="wk512")
                nc.vector.tensor_mul(sq[:], src[:, e, :], src[:, e, :])
                nc.tensor.matmul(ps_sq[:], ones_col_f32, sq[:],
                                 start=(e == 0), stop=(e == NE - 1))
            # all stat vectors at partition base 0 (walrus: 2-input SBUF
            # DVE ops need equal base partitions)
            mu = stat.tile([1, TL], F32, tag="v1")
            nc.vector.tensor_scalar_mul(mu[:], ps_sum[:], 1.0 / E)
            var = stat.tile([1, TL], F32, tag="v1")
            nc.vector.tensor_scalar_mul(var[:], ps_sq[:], 1.0 / E)
            mu2 = stat.tile([1, TL], F32, tag="v1")
            nc.vector.tensor_mul(mu2[:], mu[:], mu[:])
            nc.vector.tensor_sub(var[:], var[:], mu2[:])
            nc.vector.tensor_scalar_add(var[:], var[:], EPS)
            rec = stat.tile([1, TL], F32, tag="v1")
            nc.vector.reciprocal(rec[:], var[:])
            rstd = stat.tile([1, TL], F32, tag="v1")
            nc.scalar.sqrt(rstd[:], rec[:])
            ps_rstd = psA.tile([P, PSW], F32, tag="ps")
            nc.tensor.matmul(ps_rstd[:, :TL], ones_row_f32, rstd[:])
            cc = stat.tile([1, TL], F32, tag="v1")
            nc.vector.tensor_mul(cc[:], mu[:], rstd[:])
            ps_c = psA.tile([P, PSW], F32, tag="ps")
            nc.tensor.matmul(ps_c[:, :TL], ones_row_f32, cc[:])
            b_rstd = work.tile([P, TL], F32, tag="wk512")
            nc.vector.tensor_copy(b_rstd[:], ps_rstd[:, :TL])
            b_c = work.tile([P, TL], F32, tag="wk512")
            nc.vector.tensor_copy(b_c[:], ps_c[:, :TL])
            for e in range(NE):
                t = work.tile([P, TL], F32, tag="wk512")
                nc.vector.tensor_mul(t[:], src[:, e, :], b_rstd[:])
                nc.vector.tensor_sub(t[:], t[:], b_c[:])
                nc.vector.tensor_scalar(out_t[:, e, :], t[:], g_of_e(e), b_of_e(e),
                                        mult, add)

        def stream_w(dram_slice_of_half, n_halves):
            """yield (half, wc) chunks [P, NE, WCH] of a [P, NE, E]-like weight"""
            for half in range(n_halves):
                wc = wpool.tile([P, NE, WCH], BF16, tag="wchunk")
                nc.sync.dma_start(wc[:], dram_slice_of_half(half))
                yield half, wc

        # ================================================================
        # transformer layers
        # ================================================================
        wqkvp_d = io["wqkvp"]   # [L, 4, P, NE, E]
        fw1_d = io["fw1"]       # [L, P, NE, FF]
        fw2_d = io["fw2"]       # [L, NE, P, NF, P]

        for l in range(cfg.L):
            # ---- LN1 ---------------------------------------------------
            h_sb = hpool.tile([P, NE, TL], BF16, tag="hatt")
            layernorm(x_sb, lambda e, _l=l: lv_ap(_l, 0, e),
                      lambda e, _l=l: lv_ap(_l, 1, e), h_sb)

            # ---- k ------------------------------------------------------
            k_loc = qkvp.tile([P, NE, TL], BF16, tag="kq")
            for half, wc in stream_w(
                    lambda hf, _l=l: wqkvp_d[_l, 1, :, :, hf * WCH:(hf + 1) * WCH],
                    E // WCH):
                for j in range(WCH // P):
                    m = half * (WCH // P) + j
                    ps = psA.tile([P, PSW], F32, tag="ps")
                    for e in range(NE):
                        nc.tensor.matmul(ps[:, :TL], wc[:, e, j * P:(j + 1) * P],
                                         h_sb[:, e, :],
                                         start=(e == 0), stop=(e == NE - 1))
                    nc.vector.tensor_copy(k_loc[:, m, :], ps[:, :TL])
            kag_in = dramp.tile([E, TL], BF16, tag="kag_in")
            kag_out = dramp.tile([2 * E, TL], BF16, tag="kag_out")
            nc.sync.dma_start(kag_in[:].rearrange("(et p) t -> p et t", p=P), k_loc[:])
            nc.gpsimd.collective_compute(
                "AllGather", mybir.AluOpType.bypass, replica_groups=PAIRS,
                ins=[kag_in.opt()], outs=[kag_out.opt()])

            # ---- v (token-major) ----------------------------------------
            for half, wc in stream_w(
                    lambda hf, _l=l: wqkvp_d[_l, 2, :, :, hf * WCH:(hf + 1) * WCH],
                    E // WCH):
                # this chunk covers head-columns [half*WCH, half*WCH+WCH)
                for mt in range(NT):
                    ps = psA.tile([P, PSW], F32, tag="ps")
                    for e in range(NE):
                        nc.tensor.matmul(ps[:, :WCH], h_sb[:, e, mt * P:(mt + 1) * P],
                                         wc[:, e, :],
                                         start=(e == 0), stop=(e == NE - 1))
                    nh_here = WCH // D
                    h0 = (half * WCH) // D
                    src2 = ps[:, :WCH].rearrange("p (hh c) -> p hh c", c=D)
                    nc.vector.tensor_copy(v4[:, mt, h0:h0 + nh_here, 0:D], src2)
            vag_in = dramp.tile([TL, NH * VW], BF16, tag="vag_in")
            vag_out = dramp.tile([2 * TL, NH * VW], BF16, tag="vag_out")
            nc.sync.dma_start(vag_in[:].rearrange("(mt p) c -> p mt c", p=P), v_loc[:])
            nc.gpsimd.collective_compute(
                "AllGather", mybir.AluOpType.bypass, replica_groups=PAIRS,
                ins=[vag_in.opt()], outs=[vag_out.opt()])

            # ---- q ------------------------------------------------------
            q_sb = qkvp.tile([P, NE, TL], BF16, tag="kq")
            for half, wc in stream_w(
                    lambda hf, _l=l: wqkvp_d[_l, 0, :, :, hf * WCH:(hf + 1) * WCH],
                    E // WCH):
                for j in range(WCH // P):
                    m = half * (WCH // P) + j
                    ps = psA.tile([P, PSW], F32, tag="ps")
                    for e in range(NE):
                        nc.tensor.matmul(ps[:, :TL], wc[:, e, j * P:(j + 1) * P],
                                         h_sb[:, e, :],
                                         start=(e == 0), stop=(e == NE - 1))
                    nc.vector.tensor_copy(q_sb[:, m, :], ps[:, :TL])

            # ---- gather k/v from the pair -------------------------------
            k_all = kvall.tile([P, 2 * NE, TL], BF16, tag="kall")
            nc.sync.dma_start(k_all[:], kag_out[:].rearrange("(g p) t -> p g t", p=P))
            v_all = kvall.tile([P, NKV, NH * VW], BF16, tag="vall")
            nc.sync.dma_start(v_all[:], vag_out[:].rearrange("(g p) c -> p g c", p=P))

            # ---- attention ---------------------------------------------
            att_sb = hpool.tile([P, NE, TL], BF16, tag="hatt")
            for h in range(NH):
                pb = D * (h % 2)
                hpt = h // 2
                ps_att = psH.tile([VW, TL], F32, tag="att")
                for c in range(NKV):
                    half = c // NT
                    j = c % NT
                    lhsT_k = k_all[pb:pb + D, half * NE + hpt, j * P:(j + 1) * P]
                    rhs_q = q_sb[pb:pb + D, hpt, :]
                    ps_s = psA.tile([P, PSW], F32, tag="ps")
                    nc.tensor.matmul(ps_s[:, :TL], lhsT_k, rhs_q)
                    p_t = work.tile([P, TL], BF16, tag="wk512")
                    nc.scalar.activation(p_t[:], ps_s[:, :TL],
                                         mybir.ActivationFunctionType.Exp)
                    p_m = work.tile([P, TL], BF16, tag="wk512")
                    nc.vector.tensor_mul(p_m[:], p_t[:], mask_sb[:, c, :])
                    nc.tensor.matmul(ps_att[:], v_all[:, c, h * VW:(h + 1) * VW],
                                     p_m[:], start=(c == 0), stop=(c == NKV - 1))
                den = stat.tile([1, TL], F32, tag="v1")
                nc.vector.tensor_copy(den[:], ps_att[D:VW, :])
                rden = stat.tile([1, TL], F32, tag="v1")
                nc.vector.reciprocal(rden[:], den[:])
                ps_b = psA.tile([P, PSW], F32, tag="ps")
                nc.tensor.matmul(ps_b[:D, :TL], ones_row_f32[:, :D], rden[:])
                b_r = work.tile([P, TL], F32, tag="wk512")
                nc.vector.tensor_copy(b_r[:D, :], ps_b[:D, :TL])
                nc.vector.tensor_mul(att_sb[pb:pb + D, hpt, :], ps_att[0:D, :],
                                     b_r[:D, :])

            # ---- proj + residual ---------------------------------------
            for half, wc in stream_w(
                    lambda hf, _l=l: wqkvp_d[_l, 3, :, :, hf * WCH:(hf + 1) * WCH],
                    E // WCH):
                for j in range(WCH // P):
                    eo = half * (WCH // P) + j
                    ps = psA.tile([P, PSW], F32, tag="ps")
                    for kd in range(NE):
                        nc.tensor.matmul(ps[:, :TL], wc[:, kd, j * P:(j + 1) * P],
                                         att_sb[:, kd, :],
                                         start=(kd == 0), stop=(kd == NE - 1))
                    nc.vector.scalar_tensor_tensor(
                        x_sb[:, eo, :], ps[:, :TL], lv_ap(l, 4, eo),
                        x_sb[:, eo, :], add, add)

            # ---- LN2 + FFN ---------------------------------------------
            h_sb = hpool.tile([P, NE, TL], BF16, tag="hatt")
            layernorm(x_sb, lambda e, _l=l: lv_ap(_l, 2, e),
                      lambda e, _l=l: lv_ap(_l, 3, e), h_sb)

            relu_sb = relup.tile([P, NF, TL], BF16, tag="bigbuf")
            for ch in range(FF // WCH):
                wc = wpool.tile([P, NE, WCH], BF16, tag="wchunk")
                nc.sync.dma_start(wc[:], fw1_d[l, :, :, ch * WCH:(ch + 1) * WCH])
                for j in range(WCH // P):
                    f = ch * (WCH // P) + j
                    ps = psA.tile([P, PSW], F32, tag="ps")
                    for e in range(NE):
                        nc.tensor.matmul(ps[:, :TL], wc[:, e, j * P:(j + 1) * P],
                                         h_sb[:, e, :],
                                         start=(e == 0), stop=(e == NE - 1))
                    nc.scalar.activation(relu_sb[:, f, :], ps[:, :TL],
                                         mybir.ActivationFunctionType.Relu,
                                         bias=fb1_ap(l, f), scale=1.0)
            for eo in range(NE):
                w2c = wpool.tile([P, NF, P], BF16, tag="wchunk")
                nc.sync.dma_start(w2c[:], fw2_d[l, eo])
                ps = psA.tile([P, PSW], F32, tag="ps")
                for f in range(NF):
                    nc.tensor.matmul(ps[:, :TL], w2c[:, f, :], relu_sb[:, f, :],
                                     start=(f == 0), stop=(f == NF - 1))
                nc.vector.scalar_tensor_tensor(
                    x_sb[:, eo, :], ps[:, :TL], lv_ap(l, 5, eo),
                    x_sb[:, eo, :], add, add)

        # ================================================================
        # final LN + 8-way gather + lm head
        # ================================================================
        xout = hpool.tile([P, NE, TL], BF16, tag="hatt")
        layernorm(x_sb, lambda e: lnf_ap(0, e), lambda e: lnf_ap(1, e), xout)
        xag_in = dramp.tile([E, TL], BF16, tag="xag_in")
        xag_out = dramp.tile([NCORES * E, TL], BF16, tag="xag_out")
        nc.sync.dma_start(xag_in[:].rearrange("(et p) t -> p et t", p=P), xout[:])
        nc.gpsimd.collective_compute(
            "AllGather", mybir.AluOpType.bypass, replica_groups=ALLC,
            ins=[xag_in.opt()], outs=[xag_out.opt()])

        logits_d = io["logits"]   # [VSH, BT] f32
        lmw_d = io["lmw"]         # [P, NE, VSH] bf16
        n_lm_ch = (VSH + WCH - 1) // WCH
        for tb in range(NB):
            xtb = relup.tile([P, NE, TL], BF16, tag="bigbuf")
            nc.sync.dma_start(
                xtb[:],
                xag_out[tb * E:(tb + 1) * E, :].rearrange("(et p) t -> p et t", p=P))
            ps_se = psS.tile([1, TL], F32, tag="st")
            for ch in range(n_lm_ch):
                cw = min(WCH, VSH - ch * WCH)
                wc = wpool.tile([P, NE, WCH], BF16, tag="wchunk")
                nc.sync.dma_start(wc[:, :, :cw],
                                  lmw_d[:, :, ch * WCH:ch * WCH + cw])
                for j in range(cw // P):
                    vt = ch * (WCH // P) + j
                    ps = psA.tile([P, PSW], F32, tag="ps")
                    for e in range(NE):
                        nc.tensor.matmul(ps[:, :TL], wc[:, e, j * P:(j + 1) * P],
                                         xtb[:, e, :],
                                         start=(e == 0), stop=(e == NE - 1))
                    lg = work.tile([P, TL], F32, tag="wk512")
                    nc.vector.tensor_scalar_add(lg[:], ps[:, :TL], lmb_ap(vt))
                    nc.sync.dma_start(
                        logits_d[vt * P:(vt + 1) * P, tb * TL:(tb + 1) * TL], lg[:])
                    ex = work.tile([P, TL], BF16, tag="wk512")
                    nc.scalar.activation(ex[:], ps[:, :TL],
                                         mybir.ActivationFunctionType.Exp,
                                         bias=lmb_ap(vt), scale=1.0)
                    nc.tensor.matmul(ps_se[:], ones_col_bf, ex[:],
                                     start=(vt == 0), stop=(vt == NV - 1))
            sec = work.tile([P, TL], F32, tag="wk512")
            nc.vector.tensor_copy(sec[0:1, :], ps_se[:])
            nc.sync.dma_start(io["sumexp"][tb:tb + 1, :], sec[0:1, :])


# --------------------------------------------------------------------------
# host side
# --------------------------------------------------------------------------

def pack_weights(inputs, cfg: Cfg):
    """Pack model weights into the device layouts (shared across cores)."""
    L, E, H, FF = cfg.L, cfg.E, cfg.H, cfg.FF
    NE, NF = cfg.NE, cfg.NF
    scale = float(E) ** -0.5

    wq = _f32t(inputs["wq"]).transpose(0, 2, 1, 3).reshape(L, E, E) * scale
    wk = _f32t(inputs["wk"]).transpose(0, 2, 1, 3).reshape(L, E, E)
    wv = _f32t(inputs["wv"]).transpose(0, 2, 1, 3).reshape(L, E, E)
    wp = _f32t(inputs["w_proj"])
    wqkvp = np.stack([wq, wk, wv, wp], axis=1)            # [L,4,E,E]
    wqkvp = wqkvp.reshape(L, 4, NE, P, E).transpose(0, 1, 3, 2, 4)
    fw1 = _f32t(inputs["ff_w1"]).reshape(L, NE, P, FF).transpose(0, 2, 1, 3)
    fw2 = _f32t(inputs["ff_w2"]).reshape(L, NF, P, NE, P).transpose(0, 3, 2, 1, 4)

    def vecpack(v):  # [E] -> [P, NE]
        return _f32t(v).reshape(NE, P).T

    lvecs = np.stack([
        np.stack([vecpack(inputs["ln1_g"][l]), vecpack(inputs["ln1_b"][l]),
                  vecpack(inputs["ln2_g"][l]), vecpack(inputs["ln2_b"][l]),
                  vecpack(inputs["b_proj"][l]), vecpack(inputs["ff_b2"][l])],
                 axis=0)
        for l in range(L)], axis=0)                        # [L,6,P,NE]
    fb1 = np.stack([_f32t(inputs["ff_b1"][l]).reshape(NF, P).T
                    for l in range(L)], axis=0)            # [L,P,NF]
    lnf = np.stack([vecpack(inputs["lnf_g"]), vecpack(inputs["lnf_b"])], axis=0)

    return {
        "wqkvp": _bf16(wqkvp),
        "fw1": _bf16(fw1),
        "fw2": _bf16(fw2),
        "lvecs": _f32t(lvecs),
        "fb1": _f32t(fb1),
        "lnf": _f32t(lnf),
    }


def pack_core_inputs(inputs, cfg: Cfg, core):
    """Per-core inputs: x0, masks, lm shard."""
    E, T, V, VSH = cfg.E, cfg.T, cfg.V, cfg.VSH
    NE, NV, NKV, TL = cfg.NE, cfg.NV, cfg.NKV, cfg.TLOC
    b, halfsel = core // 2, core % 2
    t0 = halfsel * TL

    idx = np.asarray(inputs["idx"])
    tok_emb = _f32t(inputs["tok_emb"])
    pos_emb = _f32t(inputs["pos_emb"])
    x0 = tok_emb[idx[b, t0:t0 + TL]] + pos_emb[t0:t0 + TL]   # [TL, E]
    x0 = x0.T.reshape(NE, P, TL).transpose(1, 0, 2)          # [P, NE, TL]

    kv = np.arange(cfg.T)[:, None]                            # global kv index
    q = t0 + np.arange(TL)[None, :]
    mask = (kv <= q).astype(np.float32).reshape(NKV, P, TL)

    lm_w = _f32t(inputs["lm_w"])
    lm_b = _f32t(inputs["lm_b"])
    v0 = core * VSH
    wsh = np.zeros((E, VSH), np.float32)
    bsh = np.full((VSH,), PAD_BIAS, np.float32)
    hi = min(V, v0 + VSH)
    if hi > v0:
        wsh[:, :hi - v0] = lm_w[:, v0:hi]
        bsh[:hi - v0] = lm_b[v0:hi]
    lmw = wsh.reshape(NE, P, VSH).transpose(1, 0, 2)          # [P, NE, VSH]
    lmb = bsh.reshape(NV, P).T                                 # [P, NV]

    return {
        "x0": _f32t(x0),
        "masks": _bf16(mask),
        "lmw": _bf16(lmw),
        "lmb": _f32t(lmb),
    }


def declare_io(nc, cfg: Cfg):
    NE, NF, NV, NKV, TL = cfg.NE, cfg.NF, cfg.NV, cfg.NKV, cfg.TLOC
    L, E, FF, VSH = cfg.L, cfg.E, cfg.FF, cfg.VSH
    BT = cfg.B * cfg.T
    io = {}

    def din(name, shape, dt):
        io[name] = nc.dram_tensor(name, shape, dt, kind="ExternalInput").ap()

    def dout(name, shape, dt):
        io[name] = nc.dram_tensor(name, shape, dt, kind="ExternalOutput").ap()

    din("x0", [P, NE, TL], F32)
    din("masks", [NKV, P, TL], BF16)
    din("wqkvp", [L, 4, P, NE, E], BF16)
    din("fw1", [L, P, NE, FF], BF16)
    din("fw2", [L, NE, P, NF, P], BF16)
    din("lvecs", [L, 6, P, NE], F32)
    din("fb1", [L, P, NF], F32)
    din("lnf", [2, P, NE], F32)
    din("lmw", [P, NE, VSH], BF16)
    din("lmb", [P, NV], F32)
    dout("logits", [VSH, BT], F32)
    dout("sumexp", [NCORES, TL], F32)
    return io


_COMPILED = {}
LAST_RESULTS = None


def _get_program(cfg: Cfg):
    key = cfg
    if key in _COMPILED:
        return _COMPILED[key]
    nc = bacc.Bacc("TRN2", target_bir_lowering=False, debug=False,
                   num_devices=NCORES)
    io = declare_io(nc, cfg)
    with tile.TileContext(nc) as tc:
        build_body(tc, io, cfg)
    nc.compile()
    _COMPILED[key] = nc
    return nc


def kernel(_trace=False, **inputs):
    global LAST_RESULTS
    cfg = FULL
    nc = _get_program(cfg)

    shared = pack_weights(inputs, cfg)
    in_maps = []
    for core in range(NCORES):
        m = dict(shared)
        m.update(pack_core_inputs(inputs, cfg, core))
        in_maps.append(m)

    res = bass_utils.run_bass_kernel_spmd(
        nc, in_maps, core_ids=list(range(NCORES)), trace=_trace)
    LAST_RESULTS = res

    V, BT, VSH, TL = cfg.V, cfg.B * cfg.T, cfg.VSH, cfg.TLOC
    logits_fm = np.concatenate(
        [np.asarray(res.results[c]["logits"]) for c in range(NCORES)], axis=0)
    sumexp = np.zeros((BT,), np.float64)
    for c in range(NCORES):
        sumexp += np.asarray(res.results[c]["sumexp"]).astype(np.float64).reshape(BT)

    logits = logits_fm[:V].T                                  # [BT, V] view
    targets = np.asarray(inputs["targets"]).reshape(BT)
    tl = logits_fm[targets, np.arange(BT)]                    # target logits
    loss = np.float32(np.mean(np.log(sumexp) - tl))
    return np.ascontiguousarray(logits), loss
